# revision 7
# baseline (speedup 1.0000x reference)
"""Trainium2 Bass kernel for nn_Decoder (2-layer LSTM decoder + VAE heads).

Math (mirrors the jax reference exactly, including its arg-swap quirk):
    z_cat = concat([z1, z2], -1)                      [bs, 64]
    per step t (T=20):
        g1 = z_cat @ W1 + b1 + h1 @ U1 ; h1,c1 = lstm_gates(g1, c1)
        g2 = h1 @ W2 + b2 + h2 @ U2    ; h2,c2 = lstm_gates(g2, c2)
    output   = stack of h2                            [bs, T, 256]
    x_mu     = output @ Wmu + bmu
    x_logvar = output @ Wlv + blv
    x_sample = eps * exp(0.5 * x_mu) + x_logvar

Distribution: pure data parallel — batch 8192 split as 1024 rows/core over
8 NeuronCores; weights replicated; the T=20 scan is local per core.

Device design notes:
 * Transposed layout: activations are [feature, batch] so all matmuls use
   weights in natural [K, M] (lhsT) layout; zero on-device transposes.
 * bf16 matmul inputs (fp32 PSUM accumulate): 1 cyc/row on the PE + fast
   weight load; end-to-end rel err ~5e-3 for this net.
 * All gate nonlinearities are computed as tanh:  sigmoid(x) = (1+tanh(x/2))/2,
   with the (1+y)/2 affine folded into DVE scalar_tensor_tensor ops and
   host-side weight scaling (see below).  This keeps the Scalar engine on a
   single activation-table set (tanh+exp live in one set; sigmoid does not),
   avoiding ~2.7us table reloads per step.
 * Host pre-scaling: cell state is kept as C=2c, layer-1 hidden as H1=2*h1.
   Consequently U1,W2 are pre-multiplied by 0.5, and the g-gate columns of
   all gate weights by 2 (so one tanh(0.5*x) call serves all four gates).
 * b1 is folded into the W1 matmul via an appended ones-row of z_cat.
   b2 (spec-zero) has a correct fallback path via per-partition ACT bias.
 * The batch is processed in NCH chunks of NB columns, software-pipelined
   (all chunks' L1 matmuls, then all elementwise, then L2, ...) so the PE
   always has an independent chunk to chew on during the elementwise chain.
"""

import os
import sys
import numpy as np
import ml_dtypes

if "/opt/trn_rl_repo" not in sys.path:  # harmless if axon site already provides it
    sys.path.append("/opt/trn_rl_repo")

import concourse.bacc as bacc
import concourse.tile as tile
from concourse import mybir
from concourse.bass_utils import run_bass_kernel_spmd

F32 = mybir.dt.float32
BF16 = mybir.dt.bfloat16
AF = mybir.ActivationFunctionType
ALU = mybir.AluOpType

BS, T, FDIM, ZDIM, H = 8192, 20, 80, 32, 256
NCORES = 8
B = BS // NCORES  # 1024 batch rows per core
NB = int(os.environ.get("LSTM_NB", "512"))  # batch-chunk width per core
NCH = B // NB
G4 = 4 * H  # 1024 gate columns


def _build(nb, nch, t_steps, bias2, bias_mu, bias_lv):
    """Emit + compile the per-core program."""
    nc = bacc.Bacc(
        "TRN2", target_bir_lowering=False, debug=False, num_devices=NCORES
    )

    ZC = nc.dram_tensor("zc", [2 * ZDIM + 1, B], BF16, kind="ExternalInput").ap()
    EPS = nc.dram_tensor("eps", [t_steps, FDIM, B], F32, kind="ExternalInput").ap()
    W1A = nc.dram_tensor("w1a", [2 * ZDIM + 1, G4], BF16, kind="ExternalInput").ap()
    U1 = nc.dram_tensor("u1", [H, G4], BF16, kind="ExternalInput").ap()
    W2 = nc.dram_tensor("w2", [H, G4], BF16, kind="ExternalInput").ap()
    U2 = nc.dram_tensor("u2", [H, G4], BF16, kind="ExternalInput").ap()
    WHD = nc.dram_tensor("whd", [H, 2 * FDIM], BF16, kind="ExternalInput").ap()
    if bias2:
        B2 = nc.dram_tensor("b2", [G4], F32, kind="ExternalInput").ap()
    if bias_mu or bias_lv:
        BHD = nc.dram_tensor("bhd", [3, FDIM], F32, kind="ExternalInput").ap()

    HT = nc.dram_tensor("ht", [t_steps, H, B], BF16, kind="ExternalOutput").ap()
    MUT = nc.dram_tensor("mut", [t_steps, FDIM, B], F32, kind="ExternalOutput").ap()
    LVT = nc.dram_tensor("lvt", [t_steps, FDIM, B], F32, kind="ExternalOutput").ap()
    ST = nc.dram_tensor("st", [t_steps, FDIM, B], F32, kind="ExternalOutput").ap()

    n2 = 2 * nb
    slot_banks = (4 * nb * 4 + 2047) // 2048
    ps_bufs = max(2, 8 // slot_banks)

    with tile.TileContext(nc) as tc:
        with (
            tc.tile_pool(name="wt", bufs=1) as wt,
            tc.tile_pool(name="state", bufs=1) as state,
            tc.tile_pool(name="ps", bufs=ps_bufs, space="PSUM") as ps,
            tc.tile_pool(name="sig", bufs=3) as sigp,
            tc.tile_pool(name="tmp", bufs=3) as tmpp,
            tc.tile_pool(name="hd", bufs=3) as hdp,
            tc.tile_pool(name="epi", bufs=3) as epip,
        ):
            # ---- one-time: weights straight into SBUF as bf16 ----
            def load(src_ap, shape, nm):
                r = wt.tile(shape, BF16, name=f"w_{nm}")
                nc.sync.dma_start(out=r[:], in_=src_ap)
                return r

            w1r = load(W1A, [2 * ZDIM + 1, G4], "w1")
            u1r = [load(U1[k * 128 : (k + 1) * 128, :], [128, G4], f"u1{k}") for k in range(2)]
            w2r = [load(W2[k * 128 : (k + 1) * 128, :], [128, G4], f"w2{k}") for k in range(2)]
            u2r = [load(U2[k * 128 : (k + 1) * 128, :], [128, G4], f"u2{k}") for k in range(2)]
            whr = [load(WHD[k * 128 : (k + 1) * 128, :], [128, 2 * FDIM], f"wh{k}") for k in range(2)]
            zcr = load(ZC, [2 * ZDIM + 1, B], "zc")

            b2t = None
            if bias2:
                b2t = wt.tile([128, 8], F32, name="b2t")
                for m in range(8):
                    nc.sync.dma_start(out=b2t[:, m : m + 1], in_=B2[m * 128 : (m + 1) * 128])
            bmu_t = blv_t = bmu2_t = None
            if bias_mu or bias_lv:
                bh = wt.tile([FDIM, 3], F32, name="bht")
                for r in range(3):
                    nc.sync.dma_start(out=bh[:, r : r + 1], in_=BHD[r, :])
                bmu_t, blv_t, bmu2_t = bh[:, 0:1], bh[:, 1:2], bh[:, 2:3]

            # ---- persistent per-chunk state (h tiles bf16; C = 2*c fp32) ----
            h1 = [[state.tile([128, nb], BF16, name=f"h1_{n}_{j}") for j in range(2)] for n in range(nch)]
            h2 = [[state.tile([128, nb], BF16, name=f"h2_{n}_{j}") for j in range(2)] for n in range(nch)]
            c1 = [state.tile([128, n2], F32, name=f"c1_{n}") for n in range(nch)]
            c2 = [state.tile([128, n2], F32, name=f"c2_{n}") for n in range(nch)]

            def gate_mms(t, n, lname, in_pairs, rec_w, rec_h):
                A = ps.tile([128, 4 * nb], F32, tag="ps", name=f"A_{lname}_{t}_{n}")
                Bp = ps.tile([128, 4 * nb], F32, tag="ps", name=f"B_{lname}_{t}_{n}")
                for half, pt in ((0, A), (1, Bp)):
                    for mi in range(4):
                        m = half * 4 + mi
                        sl = pt[:, mi * nb : (mi + 1) * nb]
                        mm = [(w[:, m * 128 : (m + 1) * 128], rhs) for (w, rhs) in in_pairs]
                        if t > 0:
                            mm += [
                                (rec_w[k][:, m * 128 : (m + 1) * 128], rec_h[k][:])
                                for k in range(2)
                            ]
                        for i, (lhsT, rhs) in enumerate(mm):
                            nc.tensor.matmul(
                                sl, lhsT, rhs, start=(i == 0), stop=(i == len(mm) - 1)
                            )
                return A, Bp

            def gate_elem(t, n, A, Bp, hst, cst, lname, scaled_h):
                """tanh evacuation + cell update.  yA=[yi|yf], yB=[tg|yo].
                scaled_h: produce hst = (yo+1)*tanh(0.5C) (=2h); else exact h."""
                yA = sigp.tile([128, 4 * nb], F32, tag="yA", name=f"yA_{lname}_{t}_{n}")
                yB = sigp.tile([128, 4 * nb], F32, tag="yB", name=f"yB_{lname}_{t}_{n}")
                if b2sl[lname] is None:
                    nc.scalar.activation(yA[:], A[:], AF.Tanh, scale=0.5)
                    nc.scalar.activation(yB[:], Bp[:], AF.Tanh, scale=0.5)
                else:
                    for half, (src, dst) in enumerate(((A, yA), (Bp, yB))):
                        for mi in range(4):
                            nc.scalar.activation(
                                dst[:, mi * nb : (mi + 1) * nb],
                                src[:, mi * nb : (mi + 1) * nb],
                                AF.Tanh,
                                scale=0.5,
                                bias=b2sl[lname][half * 4 + mi],
                            )
                yi, yf = yA[:, 0:n2], yA[:, n2 : 4 * nb]
                tg, yo = yB[:, 0:n2], yB[:, n2 : 4 * nb]
                if t == 0:
                    # C = 2c = (yi+1)*tanh(g)
                    nc.vector.scalar_tensor_tensor(cst[:], yi, 1.0, tg, ALU.add, ALU.mult)
                else:
                    t1 = tmpp.tile([128, n2], F32, tag="t1", name=f"t1_{lname}_{t}_{n}")
                    u = tmpp.tile([128, n2], F32, tag="u", name=f"u_{lname}_{t}_{n}")
                    yf1 = tmpp.tile([128, n2], F32, tag="yf1", name=f"yf1_{lname}_{t}_{n}")
                    nc.vector.scalar_tensor_tensor(t1[:], yi, 1.0, tg, ALU.add, ALU.mult)
                    # gpsimd has no fused scalar_tensor_tensor -> two ops
                    nc.gpsimd.tensor_scalar_add(yf1[:], yf, 1.0)
                    nc.gpsimd.tensor_mul(u[:], yf1[:], cst[:])
                    nc.vector.scalar_tensor_tensor(cst[:], u[:], 0.5, t1[:], ALU.mult, ALU.add)
                tnc = tmpp.tile([128, n2], F32, tag="tnc", name=f"tnc_{lname}_{t}_{n}")
                nc.scalar.activation(tnc[:], cst[:], AF.Tanh, scale=0.5)
                if scaled_h:
                    for j in range(2):
                        nc.vector.scalar_tensor_tensor(
                            hst[j][:],
                            yo[:, j * nb : (j + 1) * nb], 1.0,
                            tnc[:, j * nb : (j + 1) * nb],
                            ALU.add, ALU.mult,
                        )
                else:
                    tmp = tmpp.tile([128, n2], F32, tag="h2t", name=f"h2t_{lname}_{t}_{n}")
                    nc.vector.scalar_tensor_tensor(
                        tmp[:], yo, 1.0, tnc[:], ALU.add, ALU.mult
                    )
                    for j in range(2):
                        nc.vector.tensor_scalar_mul(
                            hst[j][:], tmp[:, j * nb : (j + 1) * nb], 0.5
                        )

            b2sl = {"l1": None, "l2": [b2t[:, m : m + 1] for m in range(8)] if bias2 else None}

            for t in range(t_steps):
                l1ab = []
                for n in range(nch):
                    zsl = zcr[:, n * nb : (n + 1) * nb]
                    l1ab.append(gate_mms(t, n, "l1", [(w1r, zsl)], u1r, h1[n]))
                for n in range(nch):
                    A, Bp = l1ab[n]
                    gate_elem(t, n, A, Bp, h1[n], c1[n], "l1", scaled_h=True)
                l2ab = []
                for n in range(nch):
                    l2ab.append(gate_mms(
                        t, n, "l2",
                        [(w2r[0], h1[n][0][:]), (w2r[1], h1[n][1][:])],
                        u2r, h2[n],
                    ))
                for n in range(nch):
                    A2, B2p = l2ab[n]
                    gate_elem(t, n, A2, B2p, h2[n], c2[n], "l2", scaled_h=False)
                for n in range(nch):
                    zr = slice(n * nb, (n + 1) * nb)
                    # ---- heads ----
                    Hp = ps.tile([FDIM, n2], F32, tag="ps", name=f"H_{t}_{n}")
                    for col, off in ((0, 0), (1, FDIM)):
                        for k in range(2):
                            nc.tensor.matmul(
                                Hp[:, col * nb : (col + 1) * nb],
                                whr[k][:, off : off + FDIM],
                                h2[n][k][:],
                                start=(k == 0),
                                stop=(k == 1),
                            )
                    E = hdp.tile([FDIM, nb], F32, tag="E", name=f"E_{t}_{n}")
                    if bias_mu:
                        nc.scalar.activation(E[:], Hp[:, 0:nb], AF.Exp, scale=0.5, bias=bmu2_t)
                    else:
                        nc.scalar.activation(E[:], Hp[:, 0:nb], AF.Exp, scale=0.5)
                    mlv = hdp.tile([FDIM, n2], F32, tag="mlv", name=f"mlv_{t}_{n}")
                    if bias_mu or bias_lv:
                        nc.scalar.activation(mlv[:, 0:nb], Hp[:, 0:nb], AF.Identity, bias=bmu_t)
                        nc.scalar.activation(mlv[:, nb:n2], Hp[:, nb:n2], AF.Identity, bias=blv_t)
                    else:
                        nc.vector.tensor_copy(mlv[:], Hp[:])
                    ep = epip.tile([FDIM, nb], F32, tag="ep", name=f"ep_{t}_{n}")
                    nc.sync.dma_start(out=ep[:], in_=EPS[t, :, zr])
                    sm = epip.tile([FDIM, nb], F32, tag="sm", name=f"sm_{t}_{n}")
                    nc.gpsimd.tensor_mul(sm[:], ep[:], E[:])
                    ss = epip.tile([FDIM, nb], F32, tag="ss", name=f"ss_{t}_{n}")
                    nc.vector.tensor_add(ss[:], sm[:], mlv[:, nb:n2])
                    # ---- stores ----
                    for j in range(2):
                        nc.sync.dma_start(
                            out=HT[t, j * 128 : (j + 1) * 128, zr], in_=h2[n][j][:]
                        )
                    nc.sync.dma_start(out=MUT[t, :, zr], in_=mlv[:, 0:nb])
                    nc.sync.dma_start(out=LVT[t, :, zr], in_=mlv[:, nb:n2])
                    nc.sync.dma_start(out=ST[t, :, zr], in_=ss[:])

    nc.compile()
    return nc


_cache = {}


def _get_program(key):
    if key not in _cache:
        _cache[key] = _build(*key)
    return _cache[key]


def prep_weights(W1, U1, b1, W2, U2, Wmu, Wlv):
    """Host-side weight prep for the scaled-tanh formulation (see header)."""
    g = slice(2 * H, 3 * H)  # g-gate columns
    w1a = np.vstack([W1, b1[None, :]]).astype(np.float32).copy()
    w1a[:, g] *= 2.0
    u1 = (U1 * 0.5).astype(np.float32)
    u1[:, g] *= 2.0
    w2 = (W2 * 0.5).astype(np.float32)
    w2[:, g] *= 2.0
    u2 = U2.astype(np.float32).copy()
    u2[:, g] *= 2.0
    whd = np.hstack([Wmu, Wlv]).astype(np.float32)
    cvt = lambda a: a.astype(ml_dtypes.bfloat16)
    return cvt(w1a), cvt(u1), cvt(w2), cvt(u2), cvt(whd)


def run_full(inputs, trace=False, **spmd_kwargs):
    """Run the full problem on 8 cores.  Returns ((output, x_mu, x_logvar,
    x_sample), BassKernelResults)."""
    z1 = np.asarray(inputs["z1"], np.float32)
    z2 = np.asarray(inputs["z2"], np.float32)
    eps = np.asarray(inputs["eps"], np.float32)
    W1 = np.asarray(inputs["W1"], np.float32)
    U1 = np.asarray(inputs["U1"], np.float32)
    b1 = np.asarray(inputs["b1"], np.float32)
    W2 = np.asarray(inputs["W2"], np.float32)
    U2 = np.asarray(inputs["U2"], np.float32)
    b2 = np.asarray(inputs["b2"], np.float32)
    Wmu = np.asarray(inputs["Wmu"], np.float32)
    bmu = np.asarray(inputs["bmu"], np.float32)
    Wlv = np.asarray(inputs["Wlv"], np.float32)
    blv = np.asarray(inputs["blv"], np.float32)

    bias2 = bool(np.any(b2))
    bias_mu = bool(np.any(bmu))
    bias_lv = bool(np.any(blv))
    nc = _get_program((NB, NCH, T, bias2, bias_mu, bias_lv))

    w1a, u1, w2, u2, whd = prep_weights(W1, U1, b1, W2, U2, Wmu, Wlv)
    base = {"w1a": w1a, "u1": u1, "w2": w2, "u2": u2, "whd": whd}
    if bias2:
        b2eff = 0.5 * b2
        b2eff[2 * H : 3 * H] = b2[2 * H : 3 * H]
        base["b2"] = b2eff.astype(np.float32)
    if bias_mu or bias_lv:
        base["bhd"] = np.stack([bmu, blv, 0.5 * bmu]).astype(np.float32)

    in_maps = []
    for c in range(NCORES):
        rows = slice(c * B, (c + 1) * B)
        m = dict(base)
        m["zc"] = make_zc(z1[rows], z2[rows])
        m["eps"] = np.ascontiguousarray(eps[rows].transpose(1, 2, 0))
        in_maps.append(m)

    res = run_bass_kernel_spmd(
        nc, in_maps, list(range(NCORES)), trace=trace, **spmd_kwargs
    )

    output = np.empty((BS, T, H), np.float32)
    x_mu = np.empty((BS, T, FDIM), np.float32)
    x_lv = np.empty((BS, T, FDIM), np.float32)
    x_s = np.empty((BS, T, FDIM), np.float32)
    for c in range(NCORES):
        rows = slice(c * B, (c + 1) * B)
        r = res.results[c]
        output[rows] = r["ht"].astype(np.float32).transpose(2, 0, 1)
        x_mu[rows] = r["mut"].transpose(2, 0, 1)
        x_lv[rows] = r["lvt"].transpose(2, 0, 1)
        x_s[rows] = r["st"].transpose(2, 0, 1)
    return (output, x_mu, x_lv, x_s), res


def make_zc(z1r, z2r):
    zc = np.empty((2 * ZDIM + 1, z1r.shape[0]), np.float32)
    zc[0:ZDIM] = z1r.T
    zc[ZDIM : 2 * ZDIM] = z2r.T
    zc[2 * ZDIM] = 1.0
    return zc.astype(ml_dtypes.bfloat16)


def kernel(**inputs):
    return run_full(inputs, trace=False)[0]


# revision 9
# speedup vs baseline: 1.5179x; 1.5179x over previous
"""Trainium2 Bass kernel for nn_Decoder (2-layer LSTM decoder + VAE heads).

Math (mirrors the jax reference exactly, including its arg-swap quirk):
    z_cat = concat([z1, z2], -1)                      [bs, 64]
    per step t (T=20):
        g1 = z_cat @ W1 + b1 + h1 @ U1 ; h1,c1 = lstm_gates(g1, c1)
        g2 = h1 @ W2 + b2 + h2 @ U2    ; h2,c2 = lstm_gates(g2, c2)
    output   = stack of h2                            [bs, T, 256]
    x_mu     = output @ Wmu + bmu
    x_logvar = output @ Wlv + blv
    x_sample = eps * exp(0.5 * x_mu) + x_logvar

Distribution: pure data parallel — batch 8192 split as 1024 rows/core over
8 NeuronCores; weights replicated; the T=20 scan is local per core.

Device design notes:
 * Transposed layout: activations are [feature, batch] so all matmuls use
   weights in natural [K, M] (lhsT) layout; zero on-device transposes.
 * bf16 matmul inputs (fp32 PSUM accumulate): 1 cyc/row on the PE + fast
   weight load; end-to-end rel err ~5e-3 for this net.
 * All gate nonlinearities are computed as tanh:  sigmoid(x) = (1+tanh(x/2))/2,
   with the (1+y)/2 affine folded into DVE scalar_tensor_tensor ops and
   host-side weight scaling (see below).  This keeps the Scalar engine on a
   single activation-table set (tanh+exp live in one set; sigmoid does not),
   avoiding ~2.7us table reloads per step.
 * Host pre-scaling: cell state is kept as C=2c, layer-1 hidden as H1=2*h1.
   Consequently U1,W2 are pre-multiplied by 0.5, and the g-gate columns of
   all gate weights by 2 (so one tanh(0.5*x) call serves all four gates).
 * b1 is folded into the W1 matmul via an appended ones-row of z_cat.
   b2 (spec-zero) has a correct fallback path via per-partition ACT bias.
 * The batch is processed in NCH chunks of NB columns, software-pipelined
   (all chunks' L1 matmuls, then all elementwise, then L2, ...) so the PE
   always has an independent chunk to chew on during the elementwise chain.
"""

import os
import sys
import numpy as np
import ml_dtypes

if "/opt/trn_rl_repo" not in sys.path:  # harmless if axon site already provides it
    sys.path.append("/opt/trn_rl_repo")

import concourse.bacc as bacc
import concourse.tile as tile
from concourse import mybir
from concourse.bass_utils import run_bass_kernel_spmd

F32 = mybir.dt.float32
BF16 = mybir.dt.bfloat16
AF = mybir.ActivationFunctionType
ALU = mybir.AluOpType

BS, T, FDIM, ZDIM, H = 8192, 20, 80, 32, 256
NCORES = 8
B = BS // NCORES  # 1024 batch rows per core
NB = int(os.environ.get("LSTM_NB", "512"))  # batch-chunk width per core
NCH = B // NB
G4 = 4 * H  # 1024 gate columns


def _build(nb, nch, t_steps, bias2, bias_mu, bias_lv):
    """Emit + compile the per-core program."""
    nc = bacc.Bacc(
        "TRN2", target_bir_lowering=False, debug=False, num_devices=NCORES
    )

    ZC = nc.dram_tensor("zc", [2 * ZDIM + 1, B], BF16, kind="ExternalInput").ap()
    EPS = nc.dram_tensor("eps", [t_steps, FDIM, B], F32, kind="ExternalInput").ap()
    W1A = nc.dram_tensor("w1a", [2 * ZDIM + 1, G4], BF16, kind="ExternalInput").ap()
    U1 = nc.dram_tensor("u1", [H, G4], BF16, kind="ExternalInput").ap()
    W2 = nc.dram_tensor("w2", [H, G4], BF16, kind="ExternalInput").ap()
    U2 = nc.dram_tensor("u2", [H, G4], BF16, kind="ExternalInput").ap()
    WHD = nc.dram_tensor("whd", [H, 2 * FDIM], BF16, kind="ExternalInput").ap()
    if bias2:
        B2 = nc.dram_tensor("b2", [G4], F32, kind="ExternalInput").ap()
    if bias_mu or bias_lv:
        BHD = nc.dram_tensor("bhd", [3, FDIM], F32, kind="ExternalInput").ap()

    HT = nc.dram_tensor("ht", [t_steps, H, B], BF16, kind="ExternalOutput").ap()
    MUT = nc.dram_tensor("mut", [t_steps, FDIM, B], F32, kind="ExternalOutput").ap()
    LVT = nc.dram_tensor("lvt", [t_steps, FDIM, B], F32, kind="ExternalOutput").ap()
    ST = nc.dram_tensor("st", [t_steps, FDIM, B], F32, kind="ExternalOutput").ap()

    n2 = 2 * nb
    slot_banks = (4 * nb * 4 + 2047) // 2048
    ps_bufs = max(2, 8 // slot_banks)

    with tile.TileContext(nc) as tc:
        with (
            tc.tile_pool(name="wt", bufs=1) as wt,
            tc.tile_pool(name="state", bufs=1) as state,
            tc.tile_pool(name="ps", bufs=ps_bufs, space="PSUM") as ps,
            tc.tile_pool(name="sig", bufs=3) as sigp,
            tc.tile_pool(name="tmp", bufs=3) as tmpp,
            tc.tile_pool(name="hd", bufs=3) as hdp,
            tc.tile_pool(name="epi", bufs=3) as epip,
        ):
            # ---- one-time: weights straight into SBUF as bf16 ----
            def load(src_ap, shape, nm):
                r = wt.tile(shape, BF16, name=f"w_{nm}")
                nc.sync.dma_start(out=r[:], in_=src_ap)
                return r

            w1r = load(W1A, [2 * ZDIM + 1, G4], "w1")
            u1r = [load(U1[k * 128 : (k + 1) * 128, :], [128, G4], f"u1{k}") for k in range(2)]
            w2r = [load(W2[k * 128 : (k + 1) * 128, :], [128, G4], f"w2{k}") for k in range(2)]
            u2r = [load(U2[k * 128 : (k + 1) * 128, :], [128, G4], f"u2{k}") for k in range(2)]
            whr = [load(WHD[k * 128 : (k + 1) * 128, :], [128, 2 * FDIM], f"wh{k}") for k in range(2)]
            zcr = load(ZC, [2 * ZDIM + 1, B], "zc")

            b2t = None
            if bias2:
                b2t = wt.tile([128, 8], F32, name="b2t")
                for m in range(8):
                    nc.sync.dma_start(out=b2t[:, m : m + 1], in_=B2[m * 128 : (m + 1) * 128])
            bmu_t = blv_t = bmu2_t = None
            if bias_mu or bias_lv:
                bh = wt.tile([FDIM, 3], F32, name="bht")
                for r in range(3):
                    nc.sync.dma_start(out=bh[:, r : r + 1], in_=BHD[r, :])
                bmu_t, blv_t, bmu2_t = bh[:, 0:1], bh[:, 1:2], bh[:, 2:3]

            # ---- persistent per-chunk state (h tiles bf16; C = 2*c fp32) ----
            h1 = [[state.tile([128, nb], BF16, name=f"h1_{n}_{j}") for j in range(2)] for n in range(nch)]
            h2 = [[state.tile([128, nb], BF16, name=f"h2_{n}_{j}") for j in range(2)] for n in range(nch)]
            c1 = [state.tile([128, n2], F32, name=f"c1_{n}") for n in range(nch)]
            c2 = [state.tile([128, n2], F32, name=f"c2_{n}") for n in range(nch)]

            def gate_mms(t, n, lname, in_pairs, rec_w, rec_h):
                A = ps.tile([128, 4 * nb], F32, tag="ps", name=f"A_{lname}_{t}_{n}")
                Bp = ps.tile([128, 4 * nb], F32, tag="ps", name=f"B_{lname}_{t}_{n}")
                for half, pt in ((0, A), (1, Bp)):
                    for mi in range(4):
                        m = half * 4 + mi
                        sl = pt[:, mi * nb : (mi + 1) * nb]
                        mm = [(w[:, m * 128 : (m + 1) * 128], rhs) for (w, rhs) in in_pairs]
                        if t > 0:
                            mm += [
                                (rec_w[k][:, m * 128 : (m + 1) * 128], rec_h[k][:])
                                for k in range(2)
                            ]
                        for i, (lhsT, rhs) in enumerate(mm):
                            nc.tensor.matmul(
                                sl, lhsT, rhs, start=(i == 0), stop=(i == len(mm) - 1)
                            )
                return A, Bp

            def gate_elem(t, n, A, Bp, hst, cst, lname, h_eng):
                """tanh evacuation + cell update.  yA=[yi|yf], yB=[tg|yo],
                with tg the true tanh(g) (g columns pre-doubled on host).
                sigmoid(x) = (tanh(x/2)+1)/2 via one 2-scalar tensor_scalar.
                h_eng: engine for the h=sigma_o*tanh(c) products."""
                yA = sigp.tile([128, 4 * nb], F32, tag="yA", name=f"yA_{lname}_{t}_{n}")
                yB = sigp.tile([128, 4 * nb], F32, tag="yB", name=f"yB_{lname}_{t}_{n}")
                if b2sl[lname] is None:
                    nc.scalar.activation(yA[:], A[:], AF.Tanh, scale=0.5)
                    nc.scalar.activation(yB[:], Bp[:], AF.Tanh, scale=0.5)
                else:
                    for half, (src, dst) in enumerate(((A, yA), (Bp, yB))):
                        for mi in range(4):
                            nc.scalar.activation(
                                dst[:, mi * nb : (mi + 1) * nb],
                                src[:, mi * nb : (mi + 1) * nb],
                                AF.Tanh,
                                scale=0.5,
                                bias=b2sl[lname][half * 4 + mi],
                            )
                tg = yB[:, 0:n2]
                sif = tmpp.tile([128, 4 * nb], F32, tag="sif", name=f"sif_{lname}_{t}_{n}")
                nc.vector.tensor_scalar(sif[:], yA[:], 1.0, 0.5, ALU.add, ALU.mult)
                so = tmpp.tile([128, n2], F32, tag="so", name=f"so_{lname}_{t}_{n}")
                nc.vector.tensor_scalar(so[:], yB[:, n2 : 4 * nb], 1.0, 0.5, ALU.add, ALU.mult)
                if t == 0:
                    nc.vector.tensor_mul(cst[:], sif[:, 0:n2], tg)
                else:
                    t1 = tmpp.tile([128, n2], F32, tag="t1", name=f"t1_{lname}_{t}_{n}")
                    u = tmpp.tile([128, n2], F32, tag="u", name=f"u_{lname}_{t}_{n}")
                    nc.vector.tensor_mul(t1[:], sif[:, 0:n2], tg)
                    nc.gpsimd.tensor_mul(u[:], sif[:, n2 : 4 * nb], cst[:])
                    nc.vector.tensor_add(cst[:], u[:], t1[:])
                tnc = tmpp.tile([128, n2], F32, tag="tnc", name=f"tnc_{lname}_{t}_{n}")
                nc.scalar.activation(tnc[:], cst[:], AF.Tanh)
                for j in range(2):
                    h_eng.tensor_mul(
                        hst[j][:], so[:, j * nb : (j + 1) * nb], tnc[:, j * nb : (j + 1) * nb]
                    )

            b2sl = {"l1": None, "l2": [b2t[:, m : m + 1] for m in range(8)] if bias2 else None}

            for t in range(t_steps):
                l1ab = []
                for n in range(nch):
                    zsl = zcr[:, n * nb : (n + 1) * nb]
                    l1ab.append(gate_mms(t, n, "l1", [(w1r, zsl)], u1r, h1[n]))
                for n in range(nch):
                    A, Bp = l1ab[n]
                    gate_elem(t, n, A, Bp, h1[n], c1[n], "l1", nc.vector)
                l2ab = []
                for n in range(nch):
                    l2ab.append(gate_mms(
                        t, n, "l2",
                        [(w2r[0], h1[n][0][:]), (w2r[1], h1[n][1][:])],
                        u2r, h2[n],
                    ))
                for n in range(nch):
                    A2, B2p = l2ab[n]
                    gate_elem(t, n, A2, B2p, h2[n], c2[n], "l2", nc.gpsimd)
                for n in range(nch):
                    zr = slice(n * nb, (n + 1) * nb)
                    # ---- heads ----
                    Hp = ps.tile([FDIM, n2], F32, tag="ps", name=f"H_{t}_{n}")
                    for col, off in ((0, 0), (1, FDIM)):
                        for k in range(2):
                            nc.tensor.matmul(
                                Hp[:, col * nb : (col + 1) * nb],
                                whr[k][:, off : off + FDIM],
                                h2[n][k][:],
                                start=(k == 0),
                                stop=(k == 1),
                            )
                    E = hdp.tile([FDIM, nb], F32, tag="E", name=f"E_{t}_{n}")
                    if bias_mu:
                        nc.scalar.activation(E[:], Hp[:, 0:nb], AF.Exp, scale=0.5, bias=bmu2_t)
                    else:
                        nc.scalar.activation(E[:], Hp[:, 0:nb], AF.Exp, scale=0.5)
                    mlv = hdp.tile([FDIM, n2], F32, tag="mlv", name=f"mlv_{t}_{n}")
                    if bias_mu or bias_lv:
                        nc.scalar.activation(mlv[:, 0:nb], Hp[:, 0:nb], AF.Identity, bias=bmu_t)
                        nc.scalar.activation(mlv[:, nb:n2], Hp[:, nb:n2], AF.Identity, bias=blv_t)
                    else:
                        nc.vector.tensor_copy(mlv[:], Hp[:])
                    ep = epip.tile([FDIM, nb], F32, tag="ep", name=f"ep_{t}_{n}")
                    nc.sync.dma_start(out=ep[:], in_=EPS[t, :, zr])
                    sm = epip.tile([FDIM, nb], F32, tag="sm", name=f"sm_{t}_{n}")
                    nc.gpsimd.tensor_mul(sm[:], ep[:], E[:])
                    ss = epip.tile([FDIM, nb], F32, tag="ss", name=f"ss_{t}_{n}")
                    nc.gpsimd.tensor_add(ss[:], sm[:], mlv[:, nb:n2])
                    # ---- stores ----
                    for j in range(2):
                        nc.sync.dma_start(
                            out=HT[t, j * 128 : (j + 1) * 128, zr], in_=h2[n][j][:]
                        )
                    nc.sync.dma_start(out=MUT[t, :, zr], in_=mlv[:, 0:nb])
                    nc.sync.dma_start(out=LVT[t, :, zr], in_=mlv[:, nb:n2])
                    nc.sync.dma_start(out=ST[t, :, zr], in_=ss[:])

    nc.compile()
    return nc


_cache = {}


def _get_program(key):
    if key not in _cache:
        _cache[key] = _build(*key)
    return _cache[key]


def prep_weights(W1, U1, b1, W2, U2, Wmu, Wlv):
    """Host-side weight prep for the scaled-tanh formulation (see header)."""
    g = slice(2 * H, 3 * H)  # g-gate columns pre-doubled: tanh(0.5*(2x)) = tanh(x)
    w1a = np.vstack([W1, b1[None, :]]).astype(np.float32).copy()
    w1a[:, g] *= 2.0
    u1 = U1.astype(np.float32).copy()
    u1[:, g] *= 2.0
    w2 = W2.astype(np.float32).copy()
    w2[:, g] *= 2.0
    u2 = U2.astype(np.float32).copy()
    u2[:, g] *= 2.0
    whd = np.hstack([Wmu, Wlv]).astype(np.float32)
    cvt = lambda a: a.astype(ml_dtypes.bfloat16)
    return cvt(w1a), cvt(u1), cvt(w2), cvt(u2), cvt(whd)


def run_full(inputs, trace=False, **spmd_kwargs):
    """Run the full problem on 8 cores.  Returns ((output, x_mu, x_logvar,
    x_sample), BassKernelResults)."""
    z1 = np.asarray(inputs["z1"], np.float32)
    z2 = np.asarray(inputs["z2"], np.float32)
    eps = np.asarray(inputs["eps"], np.float32)
    W1 = np.asarray(inputs["W1"], np.float32)
    U1 = np.asarray(inputs["U1"], np.float32)
    b1 = np.asarray(inputs["b1"], np.float32)
    W2 = np.asarray(inputs["W2"], np.float32)
    U2 = np.asarray(inputs["U2"], np.float32)
    b2 = np.asarray(inputs["b2"], np.float32)
    Wmu = np.asarray(inputs["Wmu"], np.float32)
    bmu = np.asarray(inputs["bmu"], np.float32)
    Wlv = np.asarray(inputs["Wlv"], np.float32)
    blv = np.asarray(inputs["blv"], np.float32)

    bias2 = bool(np.any(b2))
    bias_mu = bool(np.any(bmu))
    bias_lv = bool(np.any(blv))
    nc = _get_program((NB, NCH, T, bias2, bias_mu, bias_lv))

    w1a, u1, w2, u2, whd = prep_weights(W1, U1, b1, W2, U2, Wmu, Wlv)
    base = {"w1a": w1a, "u1": u1, "w2": w2, "u2": u2, "whd": whd}
    if bias2:
        b2eff = 0.5 * b2
        b2eff[2 * H : 3 * H] = b2[2 * H : 3 * H]
        base["b2"] = b2eff.astype(np.float32)
    if bias_mu or bias_lv:
        base["bhd"] = np.stack([bmu, blv, 0.5 * bmu]).astype(np.float32)

    in_maps = []
    for c in range(NCORES):
        rows = slice(c * B, (c + 1) * B)
        m = dict(base)
        m["zc"] = make_zc(z1[rows], z2[rows])
        m["eps"] = np.ascontiguousarray(eps[rows].transpose(1, 2, 0))
        in_maps.append(m)

    res = run_bass_kernel_spmd(
        nc, in_maps, list(range(NCORES)), trace=trace, **spmd_kwargs
    )

    output = np.empty((BS, T, H), np.float32)
    x_mu = np.empty((BS, T, FDIM), np.float32)
    x_lv = np.empty((BS, T, FDIM), np.float32)
    x_s = np.empty((BS, T, FDIM), np.float32)
    for c in range(NCORES):
        rows = slice(c * B, (c + 1) * B)
        r = res.results[c]
        output[rows] = r["ht"].astype(np.float32).transpose(2, 0, 1)
        x_mu[rows] = r["mut"].transpose(2, 0, 1)
        x_lv[rows] = r["lvt"].transpose(2, 0, 1)
        x_s[rows] = r["st"].transpose(2, 0, 1)
    return (output, x_mu, x_lv, x_s), res


def make_zc(z1r, z2r):
    zc = np.empty((2 * ZDIM + 1, z1r.shape[0]), np.float32)
    zc[0:ZDIM] = z1r.T
    zc[ZDIM : 2 * ZDIM] = z2r.T
    zc[2 * ZDIM] = 1.0
    return zc.astype(ml_dtypes.bfloat16)


def kernel(**inputs):
    return run_full(inputs, trace=False)[0]


# revision 13
# speedup vs baseline: 2.3359x; 1.5389x over previous
"""Trainium2 Bass kernel for nn_Decoder (2-layer LSTM decoder + VAE heads).

Math (mirrors the jax reference exactly, including its arg-swap quirk):
    z_cat = concat([z1, z2], -1)                      [bs, 64]
    per step t (T=20):
        g1 = z_cat @ W1 + b1 + h1 @ U1 ; h1,c1 = lstm_gates(g1, c1)
        g2 = h1 @ W2 + b2 + h2 @ U2    ; h2,c2 = lstm_gates(g2, c2)
    output   = stack of h2                            [bs, T, 256]
    x_mu     = output @ Wmu + bmu
    x_logvar = output @ Wlv + blv
    x_sample = eps * exp(0.5 * x_mu) + x_logvar

Distribution: pure data parallel — batch 8192 split as 1024 rows/core over
8 NeuronCores; weights replicated; the T=20 scan is local per core.

Device design notes:
 * Transposed layout: activations are [feature, batch] so all matmuls use
   weights in natural [K, M] (lhsT) layout; zero on-device transposes.
 * bf16 matmul inputs (fp32 PSUM accumulate): 1 cyc/row on the PE + fast
   weight load; end-to-end rel err ~5e-3 for this net.
 * All gate nonlinearities are computed as tanh:  sigmoid(x) = (1+tanh(x/2))/2,
   with the (1+y)/2 affine folded into DVE scalar_tensor_tensor ops and
   host-side weight scaling (see below).  This keeps the Scalar engine on a
   single activation-table set (tanh+exp live in one set; sigmoid does not),
   avoiding ~2.7us table reloads per step.
 * Host pre-scaling: cell state is kept as C=2c, layer-1 hidden as H1=2*h1.
   Consequently U1,W2 are pre-multiplied by 0.5, and the g-gate columns of
   all gate weights by 2 (so one tanh(0.5*x) call serves all four gates).
 * b1 is folded into the W1 matmul via an appended ones-row of z_cat.
   b2 (spec-zero) has a correct fallback path via per-partition ACT bias.
 * The batch is processed in NCH chunks of NB columns, software-pipelined
   (all chunks' L1 matmuls, then all elementwise, then L2, ...) so the PE
   always has an independent chunk to chew on during the elementwise chain.
"""

import os
import sys
import numpy as np
import ml_dtypes

if "/opt/trn_rl_repo" not in sys.path:  # harmless if axon site already provides it
    sys.path.append("/opt/trn_rl_repo")

import concourse.bacc as bacc
import concourse.tile as tile
from concourse import mybir
from concourse.bass_utils import run_bass_kernel_spmd

F32 = mybir.dt.float32
BF16 = mybir.dt.bfloat16
AF = mybir.ActivationFunctionType
ALU = mybir.AluOpType

BS, T, FDIM, ZDIM, H = 8192, 20, 80, 32, 256
NCORES = 8
B = BS // NCORES  # 1024 batch rows per core
NB = int(os.environ.get("LSTM_NB", "512"))  # batch-chunk width per core
NCH = B // NB
G4 = 4 * H  # 1024 gate columns


def _build(nb, nch, t_steps, bias2, bias_mu, bias_lv):
    """Emit + compile the per-core program."""
    nc = bacc.Bacc(
        "TRN2", target_bir_lowering=False, debug=False, num_devices=NCORES
    )

    ZC = nc.dram_tensor("zc", [2 * ZDIM + 1, B], BF16, kind="ExternalInput").ap()
    EPS = nc.dram_tensor("eps", [t_steps, FDIM, B], F32, kind="ExternalInput").ap()
    W1A = nc.dram_tensor("w1a", [2 * ZDIM + 1, G4], BF16, kind="ExternalInput").ap()
    U1 = nc.dram_tensor("u1", [H, G4], BF16, kind="ExternalInput").ap()
    W2 = nc.dram_tensor("w2", [H, G4], BF16, kind="ExternalInput").ap()
    U2 = nc.dram_tensor("u2", [H, G4], BF16, kind="ExternalInput").ap()
    WHD = nc.dram_tensor("whd", [H, 2 * FDIM], BF16, kind="ExternalInput").ap()
    if bias2:
        B2 = nc.dram_tensor("b2", [G4], F32, kind="ExternalInput").ap()
    if bias_mu or bias_lv:
        BHD = nc.dram_tensor("bhd", [3, FDIM], F32, kind="ExternalInput").ap()

    HT = nc.dram_tensor("ht", [t_steps, H, B], BF16, kind="ExternalOutput").ap()
    MUT = nc.dram_tensor("mut", [t_steps, FDIM, B], F32, kind="ExternalOutput").ap()
    LVT = nc.dram_tensor("lvt", [t_steps, FDIM, B], F32, kind="ExternalOutput").ap()
    ST = nc.dram_tensor("st", [t_steps, FDIM, B], F32, kind="ExternalOutput").ap()

    n2 = 2 * nb
    slot_banks = (4 * nb * 4 + 2047) // 2048
    ps_bufs = max(2, 8 // slot_banks)

    with tile.TileContext(nc) as tc:
        with (
            tc.tile_pool(name="wt", bufs=1) as wt,
            tc.tile_pool(name="state", bufs=1) as state,
            tc.tile_pool(name="ps", bufs=ps_bufs, space="PSUM") as ps,
            tc.tile_pool(name="sig", bufs=3) as sigp,
            tc.tile_pool(name="tmp", bufs=3) as tmpp,
            tc.tile_pool(name="hd", bufs=3) as hdp,
            tc.tile_pool(name="epi", bufs=3) as epip,
        ):
            # ---- one-time: weights straight into SBUF as bf16 ----
            def load(src_ap, shape, nm):
                r = wt.tile(shape, BF16, name=f"w_{nm}")
                nc.sync.dma_start(out=r[:], in_=src_ap)
                return r

            w1r = load(W1A, [2 * ZDIM + 1, G4], "w1")
            u1r = [load(U1[k * 128 : (k + 1) * 128, :], [128, G4], f"u1{k}") for k in range(2)]
            w2r = [load(W2[k * 128 : (k + 1) * 128, :], [128, G4], f"w2{k}") for k in range(2)]
            u2r = [load(U2[k * 128 : (k + 1) * 128, :], [128, G4], f"u2{k}") for k in range(2)]
            whr = [load(WHD[k * 128 : (k + 1) * 128, :], [128, 2 * FDIM], f"wh{k}") for k in range(2)]
            zcr = load(ZC, [2 * ZDIM + 1, B], "zc")

            b2t = None
            if bias2:
                b2t = wt.tile([128, 8], F32, name="b2t")
                for m in range(8):
                    nc.sync.dma_start(out=b2t[:, m : m + 1], in_=B2[m * 128 : (m + 1) * 128])
            bmu_t = blv_t = bmu2_t = None
            if bias_mu or bias_lv:
                bh = wt.tile([FDIM, 3], F32, name="bht")
                for r in range(3):
                    nc.sync.dma_start(out=bh[:, r : r + 1], in_=BHD[r, :])
                bmu_t, blv_t, bmu2_t = bh[:, 0:1], bh[:, 1:2], bh[:, 2:3]

            # ---- persistent per-chunk state (h tiles bf16; C = 2*c fp32) ----
            # merged layout: [h_k0 | h_k1] column blocks; matmul rhs uses slices
            h1 = [state.tile([128, n2], BF16, name=f"h1_{n}") for n in range(nch)]
            h2 = [state.tile([128, n2], BF16, name=f"h2_{n}") for n in range(nch)]
            c1 = [state.tile([128, n2], F32, name=f"c1_{n}") for n in range(nch)]
            c2 = [state.tile([128, n2], F32, name=f"c2_{n}") for n in range(nch)]

            def gate_mms(t, n, lname, in_pairs, rec_w, rec_h):
                A = ps.tile([128, 4 * nb], F32, tag="ps", name=f"A_{lname}_{t}_{n}")
                Bp = ps.tile([128, 4 * nb], F32, tag="ps", name=f"B_{lname}_{t}_{n}")
                for half, pt in ((0, A), (1, Bp)):
                    for mi in range(4):
                        m = half * 4 + mi
                        sl = pt[:, mi * nb : (mi + 1) * nb]
                        mm = [(w[:, m * 128 : (m + 1) * 128], rhs) for (w, rhs) in in_pairs]
                        if t > 0:
                            mm += [
                                (rec_w[k][:, m * 128 : (m + 1) * 128],
                                 rec_h[:, k * nb : (k + 1) * nb])
                                for k in range(2)
                            ]
                        for i, (lhsT, rhs) in enumerate(mm):
                            nc.tensor.matmul(
                                sl, lhsT, rhs, start=(i == 0), stop=(i == len(mm) - 1)
                            )
                return A, Bp

            def gate_elem(t, n, A, Bp, hst, cst, lname, h_eng):
                """Gate nonlinearities + cell update.  sA=[sig_i|sig_f],
                B=[g|o] -> tg=tanh(g), so=sigmoid(o).  h = so*tanh(c) in one
                full-width op into the merged h tile.  h_eng picks the engine
                for the h product (DVE for layer1, GpSimd for layer2)."""
                sA = sigp.tile([128, 4 * nb], F32, tag="sA", name=f"sA_{lname}_{t}_{n}")
                tg = sigp.tile([128, n2], F32, tag="tg", name=f"tg_{lname}_{t}_{n}")
                so = sigp.tile([128, n2], F32, tag="so", name=f"so_{lname}_{t}_{n}")
                if b2sl[lname] is None:
                    nc.scalar.activation(sA[:], A[:], AF.Sigmoid)
                    nc.scalar.activation(tg[:], Bp[:, 0:n2], AF.Tanh)
                    nc.scalar.activation(so[:], Bp[:, n2 : 4 * nb], AF.Sigmoid)
                else:
                    for mi in range(4):
                        nc.scalar.activation(
                            sA[:, mi * nb : (mi + 1) * nb],
                            A[:, mi * nb : (mi + 1) * nb],
                            AF.Sigmoid, bias=b2sl[lname][mi],
                        )
                    for mi in range(2):
                        nc.scalar.activation(
                            tg[:, mi * nb : (mi + 1) * nb],
                            Bp[:, mi * nb : (mi + 1) * nb],
                            AF.Tanh, bias=b2sl[lname][4 + mi],
                        )
                        nc.scalar.activation(
                            so[:, mi * nb : (mi + 1) * nb],
                            Bp[:, n2 + mi * nb : n2 + (mi + 1) * nb],
                            AF.Sigmoid, bias=b2sl[lname][6 + mi],
                        )
                if t == 0:
                    nc.vector.tensor_mul(cst[:], sA[:, 0:n2], tg[:])
                else:
                    t1 = tmpp.tile([128, n2], F32, tag="t1", name=f"t1_{lname}_{t}_{n}")
                    u = tmpp.tile([128, n2], F32, tag="u", name=f"u_{lname}_{t}_{n}")
                    nc.vector.tensor_mul(t1[:], sA[:, 0:n2], tg[:])
                    nc.gpsimd.tensor_mul(u[:], sA[:, n2 : 4 * nb], cst[:])
                    nc.vector.tensor_add(cst[:], u[:], t1[:])
                tnc = tmpp.tile([128, n2], F32, tag="tnc", name=f"tnc_{lname}_{t}_{n}")
                nc.scalar.activation(tnc[:], cst[:], AF.Tanh)
                h_eng.tensor_mul(hst[:], so[:], tnc[:])

            b2sl = {"l1": None, "l2": [b2t[:, m : m + 1] for m in range(8)] if bias2 else None}

            def hsl(hst, k):
                return hst[:, k * nb : (k + 1) * nb]

            for t in range(t_steps):
                l1ab = []
                for n in range(nch):
                    zsl = zcr[:, n * nb : (n + 1) * nb]
                    l1ab.append(gate_mms(t, n, "l1", [(w1r, zsl)], u1r, h1[n]))
                for n in range(nch):
                    A, Bp = l1ab[n]
                    gate_elem(t, n, A, Bp, h1[n], c1[n], "l1", nc.vector)
                l2ab = []
                for n in range(nch):
                    l2ab.append(gate_mms(
                        t, n, "l2",
                        [(w2r[0], hsl(h1[n], 0)), (w2r[1], hsl(h1[n], 1))],
                        u2r, h2[n],
                    ))
                for n in range(nch):
                    A2, B2p = l2ab[n]
                    gate_elem(t, n, A2, B2p, h2[n], c2[n], "l2", nc.gpsimd)
                # ---- heads + sample, once per step over the full core batch ----
                # Hp layout: [mu(c0)|mu(c1)|...|lv(c0)|lv(c1)|...]
                Hp = ps.tile([FDIM, 2 * nch * nb], F32, tag="ps", name=f"H_{t}")
                for n in range(nch):
                    for col, off in ((0, 0), (1, FDIM)):
                        for k in range(2):
                            nc.tensor.matmul(
                                Hp[:, (col * nch + n) * nb : (col * nch + n + 1) * nb],
                                whr[k][:, off : off + FDIM],
                                hsl(h2[n], k),
                                start=(k == 0),
                                stop=(k == 1),
                            )
                E = hdp.tile([FDIM, B], F32, tag="E", name=f"E_{t}")
                if bias_mu:
                    nc.scalar.activation(E[:], Hp[:, 0:B], AF.Exp, scale=0.5, bias=bmu2_t)
                else:
                    nc.scalar.activation(E[:], Hp[:, 0:B], AF.Exp, scale=0.5)
                mlv = hdp.tile([FDIM, 2 * B], F32, tag="mlv", name=f"mlv_{t}")
                if bias_mu or bias_lv:
                    nc.scalar.activation(mlv[:, 0:B], Hp[:, 0:B], AF.Identity, bias=bmu_t)
                    nc.scalar.activation(mlv[:, B : 2 * B], Hp[:, B : 2 * B], AF.Identity, bias=blv_t)
                else:
                    nc.vector.tensor_copy(mlv[:], Hp[:])
                ep = epip.tile([FDIM, B], F32, tag="ep", name=f"ep_{t}")
                nc.sync.dma_start(out=ep[:], in_=EPS[t, :, :])
                sm = epip.tile([FDIM, B], F32, tag="sm", name=f"sm_{t}")
                nc.gpsimd.tensor_mul(sm[:], ep[:], E[:])
                ss = epip.tile([FDIM, B], F32, tag="ss", name=f"ss_{t}")
                nc.gpsimd.tensor_add(ss[:], sm[:], mlv[:, B : 2 * B])
                # ---- stores ----
                for n in range(nch):
                    zr = slice(n * nb, (n + 1) * nb)
                    for j in range(2):
                        nc.sync.dma_start(
                            out=HT[t, j * 128 : (j + 1) * 128, zr], in_=hsl(h2[n], j)
                        )
                nc.sync.dma_start(out=MUT[t, :, :], in_=mlv[:, 0:B])
                nc.sync.dma_start(out=LVT[t, :, :], in_=mlv[:, B : 2 * B])
                nc.sync.dma_start(out=ST[t, :, :], in_=ss[:])

    nc.compile()
    return nc


_cache = {}


def _get_program(key):
    if key not in _cache:
        _cache[key] = _build(*key)
    return _cache[key]


def prep_weights(W1, U1, b1, W2, U2, Wmu, Wlv):
    """Host-side weight prep for the scaled-tanh formulation (see header)."""
    w1a = np.vstack([W1, b1[None, :]]).astype(np.float32)
    u1 = U1.astype(np.float32)
    w2 = W2.astype(np.float32)
    u2 = U2.astype(np.float32)
    whd = np.hstack([Wmu, Wlv]).astype(np.float32)
    cvt = lambda a: a.astype(ml_dtypes.bfloat16)
    return cvt(w1a), cvt(u1), cvt(w2), cvt(u2), cvt(whd)


def run_full(inputs, trace=False, **spmd_kwargs):
    """Run the full problem on 8 cores.  Returns ((output, x_mu, x_logvar,
    x_sample), BassKernelResults)."""
    z1 = np.asarray(inputs["z1"], np.float32)
    z2 = np.asarray(inputs["z2"], np.float32)
    eps = np.asarray(inputs["eps"], np.float32)
    W1 = np.asarray(inputs["W1"], np.float32)
    U1 = np.asarray(inputs["U1"], np.float32)
    b1 = np.asarray(inputs["b1"], np.float32)
    W2 = np.asarray(inputs["W2"], np.float32)
    U2 = np.asarray(inputs["U2"], np.float32)
    b2 = np.asarray(inputs["b2"], np.float32)
    Wmu = np.asarray(inputs["Wmu"], np.float32)
    bmu = np.asarray(inputs["bmu"], np.float32)
    Wlv = np.asarray(inputs["Wlv"], np.float32)
    blv = np.asarray(inputs["blv"], np.float32)

    bias2 = bool(np.any(b2))
    bias_mu = bool(np.any(bmu))
    bias_lv = bool(np.any(blv))
    nc = _get_program((NB, NCH, T, bias2, bias_mu, bias_lv))

    w1a, u1, w2, u2, whd = prep_weights(W1, U1, b1, W2, U2, Wmu, Wlv)
    base = {"w1a": w1a, "u1": u1, "w2": w2, "u2": u2, "whd": whd}
    if bias2:
        base["b2"] = b2.astype(np.float32)
    if bias_mu or bias_lv:
        base["bhd"] = np.stack([bmu, blv, 0.5 * bmu]).astype(np.float32)

    in_maps = []
    for c in range(NCORES):
        rows = slice(c * B, (c + 1) * B)
        m = dict(base)
        m["zc"] = make_zc(z1[rows], z2[rows])
        m["eps"] = np.ascontiguousarray(eps[rows].transpose(1, 2, 0))
        in_maps.append(m)

    res = run_bass_kernel_spmd(
        nc, in_maps, list(range(NCORES)), trace=trace, **spmd_kwargs
    )

    output = np.empty((BS, T, H), np.float32)
    x_mu = np.empty((BS, T, FDIM), np.float32)
    x_lv = np.empty((BS, T, FDIM), np.float32)
    x_s = np.empty((BS, T, FDIM), np.float32)
    for c in range(NCORES):
        rows = slice(c * B, (c + 1) * B)
        r = res.results[c]
        output[rows] = r["ht"].astype(np.float32).transpose(2, 0, 1)
        x_mu[rows] = r["mut"].transpose(2, 0, 1)
        x_lv[rows] = r["lvt"].transpose(2, 0, 1)
        x_s[rows] = r["st"].transpose(2, 0, 1)
    return (output, x_mu, x_lv, x_s), res


def make_zc(z1r, z2r):
    zc = np.empty((2 * ZDIM + 1, z1r.shape[0]), np.float32)
    zc[0:ZDIM] = z1r.T
    zc[ZDIM : 2 * ZDIM] = z2r.T
    zc[2 * ZDIM] = 1.0
    return zc.astype(ml_dtypes.bfloat16)


def kernel(**inputs):
    return run_full(inputs, trace=False)[0]
